# revision 9
# baseline (speedup 1.0000x reference)
"""Trainium2 Bass kernel for 16-head cross attention, tensor-parallel over 8 cores.

Reference computation (fp32):
    q = (x @ Wq).reshape(n, 16, 64)   # x [2048, 1024], Wq [1024, 1024]
    k = (ctx @ Wk).reshape(m, 16, 64) # ctx [2048, 768]
    v = (ctx @ Wv).reshape(m, 16, 64)
    out[h] = softmax(q[h] @ k[h].T / 8) @ v[h]
    y = out.reshape(n, 1024) @ Wo

Sharding: heads split 2-per-core (columns of Wq/Wk/Wv, rows of Wo). Each core
produces a partial y (transposed); the host sums the 8 partials.

Per-core layout choices:
  - x, ctx are fed pre-transposed (xT [1024, 2048], ctxT [768, 2048]) so the
    contraction dim of every projection matmul lands on SBUF partitions.
  - Scores are computed transposed (scoresT [m, n]) so the PV contraction (m)
    is on partitions; softmax denominators come from a ones-column appended
    to v; no max subtraction (scores ~ N(0,1), exp is safe in fp32).
  - The 1/8 softmax scale is folded into Wq on the host.
"""

import os
import sys

for _p in ("/opt/trn_rl_repo", "/root/.axon_site/_ro/trn_rl_repo"):
    if os.path.isdir(_p) and _p not in sys.path:
        sys.path.insert(0, _p)

import numpy as np
import ml_dtypes

import concourse.bass as bass
import concourse.mybir as mybir
import concourse.tile as tile
from concourse import bacc
from concourse.bass_utils import run_bass_kernel_spmd

P = 128
N_TOK = 2048  # n: query rows
M_TOK = 2048  # m: context rows
D = 1024
C = 768
HEADS = 16
DH = 64  # head dim
HPC = 2  # heads per core
SCALE = 8.0  # sqrt(DH)

NB = 512  # n-block width for the attention phase
DK = D // P  # 8 contraction chunks for x projections
CK = C // P  # 6 contraction chunks for ctx projections
MT = M_TOK // P  # 16 context chunks
NBLK = N_TOK // NB  # 4

# "bf16" | "f32r" | "f32" — f32r streams fp32 data through the PE at bf16
# rate (moving dim >= 256) with ~tf32-ish precision.
DTYPE_MODE = os.environ.get("CA_DTYPE", "f32r")


def _dtypes():
    # (storage dtype, numpy dtype, matmul dtype). For f32r the whole chain
    # must be declared float32r (BIR verifier: producers must round to f32r).
    if DTYPE_MODE == "bf16":
        return mybir.dt.bfloat16, ml_dtypes.bfloat16, mybir.dt.bfloat16
    if DTYPE_MODE == "f32r":
        return mybir.dt.float32r, np.float32, mybir.dt.float32r
    return mybir.dt.float32, np.float32, mybir.dt.float32


def _mm_cast(ap, mm_dt):
    return ap.bitcast(mm_dt) if ap.dtype != mm_dt else ap


def build_core_program():
    dt_store, _, dt_mm = _dtypes()
    f32 = mybir.dt.float32

    nc = bacc.Bacc("TRN2", target_bir_lowering=False, debug=False)

    xT = nc.declare_dram_parameter("xT", [D, N_TOK], dt_store, isOutput=False)
    ctxT = nc.declare_dram_parameter("ctxT", [C, M_TOK], dt_store, isOutput=False)
    wq = nc.declare_dram_parameter("wq", [D, P], dt_store, isOutput=False)
    wk = nc.declare_dram_parameter("wk", [C, P], dt_store, isOutput=False)
    wv = nc.declare_dram_parameter("wv", [C, P], dt_store, isOutput=False)
    wo = nc.declare_dram_parameter("wo", [P, D], dt_store, isOutput=False)
    yT = nc.declare_dram_parameter("yT", [D, N_TOK], f32, isOutput=True)

    with tile.TileContext(nc) as tc:
        with (
            tc.tile_pool(name="wts", bufs=1) as wts,
            tc.tile_pool(name="att", bufs=3) as att,
            tc.tile_pool(name="yout", bufs=3) as yout,
            tc.tile_pool(name="small", bufs=4) as small,
            tc.tile_pool(name="ps_big", bufs=2, space="PSUM") as ps_big,  # 2x2 banks
            tc.tile_pool(name="ps_pv", bufs=2, space="PSUM") as ps_pv,  # 2x1
            tc.tile_pool(name="ps_y", bufs=1, space="PSUM") as ps_y,  # 1x1
            tc.tile_pool(name="ps_bc", bufs=1, space="PSUM") as ps_bc,  # 1x1
        ):
            # ---- input DMA (ctx side first: kT and v unblock the attention) ----
            ctxT_sb = wts.tile([P, CK, M_TOK], dt_store)
            for ck in range(CK):
                nc.sync.dma_start(
                    ctxT_sb[:, ck, :], ctxT.ap()[ck * P : (ck + 1) * P, :]
                )
            wk_sb = wts.tile([P, CK, P], dt_store)
            nc.sync.dma_start(wk_sb[:], wk.ap().rearrange("(o p) e -> p o e", p=P))
            wv_sb = wts.tile([P, CK, P], dt_store)
            nc.sync.dma_start(wv_sb[:], wv.ap().rearrange("(o p) e -> p o e", p=P))
            xT_sb = wts.tile([P, DK, N_TOK], dt_store)
            for dk in range(DK):
                nc.sync.dma_start(xT_sb[:, dk, :], xT.ap()[dk * P : (dk + 1) * P, :])
            wq_sb = wts.tile([P, DK, P], dt_store)
            nc.sync.dma_start(wq_sb[:], wq.ap().rearrange("(o p) e -> p o e", p=P))
            wo_sb = wts.tile([P, D], dt_store)
            nc.sync.dma_start(wo_sb[:], wo.ap())

            # ---- persistent intermediates ----
            kT_sb = wts.tile([P, N_TOK], dt_store)  # [dk(2 heads), m]
            qT_sb = wts.tile([P, N_TOK], dt_store)  # [dq(2 heads), n]
            vA_sb = wts.tile([P, MT, DH + 1], dt_store)  # [m, mt, dv+ones]
            vB_sb = wts.tile([P, MT, DH + 1], dt_store)
            oT_sb = wts.tile([P, N_TOK], dt_store)  # attn out^T, both heads
            ones_sb = wts.tile([1, DH], mybir.dt.float32r)

            def _memset(ap, val):
                if ap.dtype == mybir.dt.float32r:
                    ap = ap.bitcast(f32)
                nc.vector.memset(ap, val)

            _memset(ones_sb[:], 1.0)
            _memset(vA_sb[:, :, DH : DH + 1], 1.0)
            _memset(vB_sb[:, :, DH : DH + 1], 1.0)

            # ---- phase 1a: kT = wk^T @ ctxT  (accumulate over ck) ----
            for nb in range(4):
                ps = ps_big.tile([P, 2, NB], f32, tag="ps_big")
                pk = ps[:, 0, :]
                for ck in range(CK):
                    nc.tensor.matmul(
                        pk,
                        _mm_cast(wk_sb[:, ck, :], dt_mm),
                        _mm_cast(ctxT_sb[:, ck, nb * NB : (nb + 1) * NB], dt_mm),
                        start=(ck == 0),
                        stop=(ck == CK - 1),
                    )
                nc.vector.tensor_copy(kT_sb[:, nb * NB : (nb + 1) * NB], pk)

            # ---- phase 1b: v natural = ctx @ wv; stationary = ctxT chunk ----
            for mt in range(MT):
                ps = ps_pv.tile([P, NB], f32, tag="ps_pv")
                pv = ps[:, :P]
                for ck in range(CK):
                    nc.tensor.matmul(
                        pv,
                        _mm_cast(ctxT_sb[:, ck, mt * P : (mt + 1) * P], dt_mm),
                        _mm_cast(wv_sb[:, ck, :], dt_mm),
                        start=(ck == 0),
                        stop=(ck == CK - 1),
                    )
                nc.vector.tensor_copy(vA_sb[:, mt, :DH], pv[:, :DH])
                nc.vector.tensor_copy(vB_sb[:, mt, :DH], pv[:, DH:])

            # ---- phase 1c: qT = wq^T @ xT ----
            for nb in range(4):
                ps = ps_big.tile([P, 2, NB], f32, tag="ps_big")
                pq = ps[:, 0, :]
                for dk in range(DK):
                    nc.tensor.matmul(
                        pq,
                        _mm_cast(wq_sb[:, dk, :], dt_mm),
                        _mm_cast(xT_sb[:, dk, nb * NB : (nb + 1) * NB], dt_mm),
                        start=(dk == 0),
                        stop=(dk == DK - 1),
                    )
                nc.vector.tensor_copy(qT_sb[:, nb * NB : (nb + 1) * NB], pq)

            # ---- phase 2: attention + output projection per n-block ----
            for nb in range(NBLK):
                nsl = slice(nb * NB, (nb + 1) * NB)
                pvA = ps_pv.tile([P, NB], f32, tag="ps_pv")
                pvB = ps_pv.tile([P, NB], f32, tag="ps_pv")
                for mt in range(MT):
                    msl = slice(mt * P, (mt + 1) * P)
                    # scoresT for both heads into one 2-bank psum tile
                    sc = ps_big.tile([P, 2, NB], f32, tag="ps_big")
                    nc.tensor.matmul(
                        sc[:, 0, :],
                        _mm_cast(kT_sb[0:DH, msl], dt_mm),
                        _mm_cast(qT_sb[0:DH, nsl], dt_mm),
                        start=True,
                        stop=True,
                    )
                    nc.tensor.matmul(
                        sc[:, 1, :],
                        _mm_cast(kT_sb[DH:P, msl], dt_mm),
                        _mm_cast(qT_sb[DH:P, nsl], dt_mm),
                        start=True,
                        stop=True,
                    )
                    # exp of both heads in one ACT op
                    at = att.tile([P, 2, NB], dt_store, tag="att")
                    nc.scalar.activation(
                        at[:], sc[:], mybir.ActivationFunctionType.Exp
                    )
                    # PV accumulation (ones column gives softmax sums in row DH)
                    nc.tensor.matmul(
                        pvA[: DH + 1, :],
                        _mm_cast(vA_sb[:, mt, :], dt_mm),
                        _mm_cast(at[:, 0, :], dt_mm),
                        start=(mt == 0),
                        stop=(mt == MT - 1),
                    )
                    nc.tensor.matmul(
                        pvB[: DH + 1, :],
                        _mm_cast(vB_sb[:, mt, :], dt_mm),
                        _mm_cast(at[:, 1, :], dt_mm),
                        start=(mt == 0),
                        stop=(mt == MT - 1),
                    )

                # normalize: oT[h] = pv[0:64] * (1/pv[64]) broadcast over rows
                for h, pv in ((0, pvA), (1, pvB)):
                    rc = small.tile([1, NB], mybir.dt.float32r, tag="recip")
                    with nc.allow_low_precision(reason="f32r recip of O(1e3) sums"):
                        nc.vector.reciprocal(rc[:], pv[DH : DH + 1, :])
                    bc = ps_bc.tile([DH, NB], f32, tag="bcast")
                    nc.tensor.matmul(bc[:], ones_sb[:], rc[:], start=True, stop=True)
                    bcs = small.tile([DH, NB], f32, tag="bcast_sb")
                    nc.vector.tensor_copy(bcs[:], bc[:])
                    nc.vector.tensor_mul(
                        oT_sb[h * DH : (h + 1) * DH, nsl], pv[:DH, :], bcs[:]
                    )

                # output projection for this n-block: yT[dout, nsl]
                for dt_i in range(8):
                    py = ps_y.tile([P, NB], f32, tag="ps_y")
                    nc.tensor.matmul(
                        py[:],
                        _mm_cast(wo_sb[:, dt_i * P : (dt_i + 1) * P], dt_mm),
                        _mm_cast(oT_sb[:, nsl], dt_mm),
                        start=True,
                        stop=True,
                    )
                    ys = yout.tile([P, NB], f32, tag="yout")
                    nc.vector.tensor_copy(ys[:], py[:])
                    nc.sync.dma_start(
                        yT.ap()[dt_i * P : (dt_i + 1) * P, nsl], ys[:]
                    )

    nc.compile()
    return nc


_NC_CACHE = {}


def _get_nc():
    key = DTYPE_MODE
    if key not in _NC_CACHE:
        _NC_CACHE[key] = build_core_program()
    return _NC_CACHE[key]


def _prep_in_maps(x, ctx, Wq, Wk, Wv, Wo):
    _, np_dt, _ = _dtypes()
    xT = np.ascontiguousarray(x.T).astype(np_dt)
    ctxT = np.ascontiguousarray(ctx.T).astype(np_dt)
    Wq_s = (Wq / SCALE).astype(np.float32)
    in_maps = []
    for cc in range(8):
        csl = slice(cc * P, (cc + 1) * P)
        in_maps.append(
            {
                "xT": xT,
                "ctxT": ctxT,
                "wq": np.ascontiguousarray(Wq_s[:, csl]).astype(np_dt),
                "wk": np.ascontiguousarray(Wk[:, csl]).astype(np_dt),
                "wv": np.ascontiguousarray(Wv[:, csl]).astype(np_dt),
                "wo": np.ascontiguousarray(Wo[csl, :]).astype(np_dt),
            }
        )
    return in_maps


def run(x, ctx, Wq, Wk, Wv, Wo, trace=False):
    nc = _get_nc()
    in_maps = _prep_in_maps(x, ctx, Wq, Wk, Wv, Wo)
    res = run_bass_kernel_spmd(nc, in_maps, core_ids=list(range(8)), trace=trace)
    acc = np.zeros((D, N_TOK), np.float32)
    for r in res.results:
        acc += r["yT"]
    return np.ascontiguousarray(acc.T), res


def kernel(x, ctx, Wq, Wk, Wv, Wo):
    x = np.asarray(x, dtype=np.float32)
    ctx = np.asarray(ctx, dtype=np.float32)
    Wq = np.asarray(Wq, dtype=np.float32)
    Wk = np.asarray(Wk, dtype=np.float32)
    Wv = np.asarray(Wv, dtype=np.float32)
    Wo = np.asarray(Wo, dtype=np.float32)
    y, _ = run(x, ctx, Wq, Wk, Wv, Wo, trace=False)
    return y


# revision 20
# speedup vs baseline: 1.3572x; 1.3572x over previous
"""Trainium2 Bass kernel for 16-head cross attention, tensor-parallel over 8 cores.

Reference computation (fp32):
    q = (x @ Wq).reshape(n, 16, 64)   # x [2048, 1024], Wq [1024, 1024]
    k = (ctx @ Wk).reshape(m, 16, 64) # ctx [2048, 768]
    v = (ctx @ Wv).reshape(m, 16, 64)
    out[h] = softmax(q[h] @ k[h].T / 8) @ v[h]
    y = out.reshape(n, 1024) @ Wo

Sharding: heads split 2-per-core (columns of Wq/Wk/Wv, rows of Wo). Each core
produces a partial y (transposed); the host sums the 8 partials.

Per-core layout choices:
  - x, ctx are fed pre-transposed (xT [1024, 2048], ctxT [768, 2048]) so the
    contraction dim of every projection matmul lands on SBUF partitions.
  - Scores are computed transposed (scoresT [m, n]) so the PV contraction (m)
    is on partitions; softmax denominators come from a ones-column appended
    to v; no max subtraction (scores ~ N(0,1), exp is safe in fp32).
  - The 1/8 softmax scale is folded into Wq on the host.
"""

import os
import sys

for _p in ("/opt/trn_rl_repo", "/root/.axon_site/_ro/trn_rl_repo"):
    if os.path.isdir(_p) and _p not in sys.path:
        sys.path.insert(0, _p)

import numpy as np
import ml_dtypes

import concourse.bass as bass
import concourse.mybir as mybir
import concourse.tile as tile
from concourse import bacc
from concourse.bass_utils import run_bass_kernel_spmd

P = 128
N_TOK = 2048  # n: query rows
M_TOK = 2048  # m: context rows
D = 1024
C = 768
HEADS = 16
DH = 64  # head dim
HPC = 2  # heads per core
SCALE = 8.0  # sqrt(DH)

NB = 512  # n-block width for the attention phase
DK = D // P  # 8 contraction chunks for x projections
CK = C // P  # 6 contraction chunks for ctx projections
MT = M_TOK // P  # 16 context chunks
NBLK = N_TOK // NB  # 4

# "bf16" | "f32r" | "f32" — f32r keeps ~tf32 precision but its weight loads
# cannot overlap with matmuls (fused 4-byte LDWEIGHTS), costing ~50% PE time.
DTYPE_MODE = os.environ.get("CA_DTYPE", "bf16")


def _dtypes():
    # (storage dtype, numpy dtype, matmul dtype). For f32r the whole chain
    # must be declared float32r (BIR verifier: producers must round to f32r).
    if DTYPE_MODE == "bf16":
        return mybir.dt.bfloat16, ml_dtypes.bfloat16, mybir.dt.bfloat16
    if DTYPE_MODE == "f32r":
        return mybir.dt.float32r, np.float32, mybir.dt.float32r
    return mybir.dt.float32, np.float32, mybir.dt.float32


def _mm_cast(ap, mm_dt):
    return ap.bitcast(mm_dt) if ap.dtype != mm_dt else ap


def build_core_program():
    dt_store, _, dt_mm = _dtypes()
    f32 = mybir.dt.float32

    nc = bacc.Bacc("TRN2", target_bir_lowering=False, debug=False)

    xT = nc.declare_dram_parameter("xT", [D, N_TOK], dt_store, isOutput=False)
    ctxT = nc.declare_dram_parameter("ctxT", [C, M_TOK], dt_store, isOutput=False)
    wq = nc.declare_dram_parameter("wq", [D, P], dt_store, isOutput=False)
    wk = nc.declare_dram_parameter("wk", [C, P], dt_store, isOutput=False)
    wv = nc.declare_dram_parameter("wv", [C, P], dt_store, isOutput=False)
    wo = nc.declare_dram_parameter("wo", [P, D], dt_store, isOutput=False)
    yT = nc.declare_dram_parameter("yT", [D, N_TOK], f32, isOutput=True)

    with tile.TileContext(nc) as tc:
        with (
            tc.tile_pool(name="wts", bufs=1) as wts,
            tc.tile_pool(name="att", bufs=3) as att,
            tc.tile_pool(name="yout", bufs=3) as yout,
            tc.tile_pool(name="small", bufs=4) as small,
            tc.tile_pool(name="ps_big", bufs=2, space="PSUM") as ps_big,  # 2x2 banks
            tc.tile_pool(name="ps_pv", bufs=2, space="PSUM") as ps_pv,  # 2x1
            tc.tile_pool(name="ps_y", bufs=2, space="PSUM") as ps_y,  # 2x1
        ):
            # ---- input DMA (ctx side first: kT and v unblock the attention) ----
            ctxT_sb = wts.tile([P, CK, M_TOK], dt_store)
            for ck in range(CK):
                nc.sync.dma_start(
                    ctxT_sb[:, ck, :], ctxT.ap()[ck * P : (ck + 1) * P, :]
                )
            # weights arrive host-pre-shuffled as [P, o, e] so the DMA is a
            # single contiguous copy (the strided rearrange cost ~15us of
            # descriptor issue on the Sync queue)
            wk_sb = wts.tile([P, CK, P], dt_store)
            nc.sync.dma_start(wk_sb[:], wk.ap().rearrange("(p o) e -> p o e", o=CK))
            wv_sb = wts.tile([P, CK, P], dt_store)
            nc.sync.dma_start(wv_sb[:], wv.ap().rearrange("(p o) e -> p o e", o=CK))
            xT_sb = wts.tile([P, DK, N_TOK], dt_store)
            for dk in range(DK):
                nc.sync.dma_start(xT_sb[:, dk, :], xT.ap()[dk * P : (dk + 1) * P, :])
            wq_sb = wts.tile([P, DK, P], dt_store)
            nc.sync.dma_start(wq_sb[:], wq.ap().rearrange("(p o) e -> p o e", o=DK))
            wo_sb = wts.tile([P, D], dt_store)
            nc.sync.dma_start(wo_sb[:], wo.ap())

            # ---- persistent intermediates ----
            kT_sb = wts.tile([P, N_TOK], dt_store)  # [dk(2 heads), m]
            qT_sb = wts.tile([P, N_TOK], dt_store)  # [dq(2 heads), n]
            vA_sb = wts.tile([P, MT, DH + 1], dt_store)  # [m, mt, dv+ones]
            vB_sb = wts.tile([P, MT, DH + 1], dt_store)
            oT_sb = wts.tile([P, N_TOK], dt_store)  # attn out^T, both heads

            def _memset(ap, val):
                if ap.dtype == mybir.dt.float32r:
                    ap = ap.bitcast(f32)
                nc.vector.memset(ap, val)

            _memset(vA_sb[:, :, DH : DH + 1], 1.0)
            _memset(vB_sb[:, :, DH : DH + 1], 1.0)

            # ---- phase 1a: kT = wk^T @ ctxT  (accumulate over ck) ----
            with nc.named_scope("ph1_kT"):
                for nb in range(4):
                    ps = ps_big.tile([P, 2, NB], f32, tag="ps_big")
                    pk = ps[:, 0, :]
                    for ck in range(CK):
                        nc.tensor.matmul(
                            pk,
                            _mm_cast(wk_sb[:, ck, :], dt_mm),
                            _mm_cast(ctxT_sb[:, ck, nb * NB : (nb + 1) * NB], dt_mm),
                            start=(ck == 0),
                            stop=(ck == CK - 1),
                        )
                    nc.vector.tensor_copy(kT_sb[:, nb * NB : (nb + 1) * NB], pk)

            # ---- phase 1b: v natural = ctx @ wv; stationary = ctxT chunk ----
            with nc.named_scope("ph1_v"):
                for mt in range(MT):
                    ps = ps_pv.tile([P, NB], f32, tag="ps_pv")
                    pv = ps[:, :P]
                    for ck in range(CK):
                        nc.tensor.matmul(
                            pv,
                            _mm_cast(ctxT_sb[:, ck, mt * P : (mt + 1) * P], dt_mm),
                            _mm_cast(wv_sb[:, ck, :], dt_mm),
                            start=(ck == 0),
                            stop=(ck == CK - 1),
                        )
                    nc.vector.tensor_copy(vA_sb[:, mt, :DH], pv[:, :DH])
                    nc.vector.tensor_copy(vB_sb[:, mt, :DH], pv[:, DH:])

            # ---- phase 1c: qT = wq^T @ xT ----
            with nc.named_scope("ph1_qT"):
                for nb in range(4):
                    ps = ps_big.tile([P, 2, NB], f32, tag="ps_big")
                    pq = ps[:, 0, :]
                    for dk in range(DK):
                        nc.tensor.matmul(
                            pq,
                            _mm_cast(wq_sb[:, dk, :], dt_mm),
                            _mm_cast(xT_sb[:, dk, nb * NB : (nb + 1) * NB], dt_mm),
                            start=(dk == 0),
                            stop=(dk == DK - 1),
                        )
                    nc.vector.tensor_copy(qT_sb[:, nb * NB : (nb + 1) * NB], pq)

            # ---- phase 2: attention + output projection per n-block ----
            for nb in range(NBLK):
                nsl = slice(nb * NB, (nb + 1) * NB)
                with nc.named_scope(f"ph2_att{nb}"):
                    pvA = ps_pv.tile([P, NB], f32, tag="ps_pv")
                    pvB = ps_pv.tile([P, NB], f32, tag="ps_pv")
                    for mt in range(MT):
                        msl = slice(mt * P, (mt + 1) * P)
                        # scoresT for both heads into one 2-bank psum tile
                        sc = ps_big.tile([P, 2, NB], f32, tag="ps_big")
                        nc.tensor.matmul(
                            sc[:, 0, :],
                            _mm_cast(kT_sb[0:DH, msl], dt_mm),
                            _mm_cast(qT_sb[0:DH, nsl], dt_mm),
                            start=True,
                            stop=True,
                        )
                        nc.tensor.matmul(
                            sc[:, 1, :],
                            _mm_cast(kT_sb[DH:P, msl], dt_mm),
                            _mm_cast(qT_sb[DH:P, nsl], dt_mm),
                            start=True,
                            stop=True,
                        )
                        # exp of both heads in one ACT op
                        at = att.tile([P, 2, NB], dt_store, tag="att")
                        nc.scalar.activation(
                            at[:], sc[:], mybir.ActivationFunctionType.Exp
                        )
                        # PV accumulation (ones column gives softmax sums, row DH)
                        nc.tensor.matmul(
                            pvA[: DH + 1, :],
                            _mm_cast(vA_sb[:, mt, :], dt_mm),
                            _mm_cast(at[:, 0, :], dt_mm),
                            start=(mt == 0),
                            stop=(mt == MT - 1),
                        )
                        nc.tensor.matmul(
                            pvB[: DH + 1, :],
                            _mm_cast(vB_sb[:, mt, :], dt_mm),
                            _mm_cast(at[:, 1, :], dt_mm),
                            start=(mt == 0),
                            stop=(mt == MT - 1),
                        )

                with nc.named_scope(f"ph2_norm{nb}"):
                    # normalize: oT[h] = pv[0:64] * (1/pv[64]) broadcast over rows
                    for h, pv in ((0, pvA), (1, pvB)):
                        # recip_approx_fast mishandles nonzero base partitions:
                        # bounce the sums row to a partition-0 SBUF tile first
                        sums_sb = small.tile([1, NB], f32, tag="sums")
                        nc.vector.tensor_copy(sums_sb[:], pv[DH : DH + 1, :])
                        rcf = small.tile([1, NB], f32, tag="recip_f32")
                        nc.vector.reciprocal_approx_fast(rcf[:], sums_sb[:])
                        bcs = small.tile([DH, NB], f32, tag="bcast_sb")
                        nc.gpsimd.partition_broadcast(bcs[:], rcf[:])
                        nc.vector.tensor_mul(
                            oT_sb[h * DH : (h + 1) * DH, nsl], pv[:DH, :], bcs[:]
                        )

                with nc.named_scope(f"ph2_proj{nb}"):
                    # output projection for this n-block: yT[dout, nsl]
                    for dt_i in range(8):
                        py = ps_y.tile([P, NB], f32, tag="ps_y")
                        nc.tensor.matmul(
                            py[:],
                            _mm_cast(wo_sb[:, dt_i * P : (dt_i + 1) * P], dt_mm),
                            _mm_cast(oT_sb[:, nsl], dt_mm),
                            start=True,
                            stop=True,
                        )
                        ys = yout.tile([P, NB], f32, tag="yout")
                        nc.vector.tensor_copy(ys[:], py[:])
                        nc.sync.dma_start(
                            yT.ap()[dt_i * P : (dt_i + 1) * P, nsl], ys[:]
                        )

    nc.compile()
    return nc


_NC_CACHE = {}


def _get_nc():
    key = DTYPE_MODE
    if key not in _NC_CACHE:
        _NC_CACHE[key] = build_core_program()
    return _NC_CACHE[key]


def _shuffle_w(w):
    # [o*P + p, e] -> [p*o_n + o, e] so each SBUF partition's rows are
    # contiguous in DRAM (single contiguous DMA into a [P, o, e] tile)
    o_n = w.shape[0] // P
    return np.ascontiguousarray(
        w.reshape(o_n, P, w.shape[1]).transpose(1, 0, 2).reshape(w.shape)
    )


def _prep_in_maps(x, ctx, Wq, Wk, Wv, Wo):
    _, np_dt, _ = _dtypes()
    xT = np.ascontiguousarray(x.T).astype(np_dt)
    ctxT = np.ascontiguousarray(ctx.T).astype(np_dt)
    Wq_s = (Wq / SCALE).astype(np.float32)
    in_maps = []
    for cc in range(8):
        csl = slice(cc * P, (cc + 1) * P)
        in_maps.append(
            {
                "xT": xT,
                "ctxT": ctxT,
                "wq": _shuffle_w(np.ascontiguousarray(Wq_s[:, csl])).astype(np_dt),
                "wk": _shuffle_w(np.ascontiguousarray(Wk[:, csl])).astype(np_dt),
                "wv": _shuffle_w(np.ascontiguousarray(Wv[:, csl])).astype(np_dt),
                "wo": np.ascontiguousarray(Wo[csl, :]).astype(np_dt),
            }
        )
    return in_maps


def run(x, ctx, Wq, Wk, Wv, Wo, trace=False):
    nc = _get_nc()
    in_maps = _prep_in_maps(x, ctx, Wq, Wk, Wv, Wo)
    res = run_bass_kernel_spmd(nc, in_maps, core_ids=list(range(8)), trace=trace)
    acc = np.zeros((D, N_TOK), np.float32)
    for r in res.results:
        acc += r["yT"]
    return np.ascontiguousarray(acc.T), res


def kernel(x, ctx, Wq, Wk, Wv, Wo):
    x = np.asarray(x, dtype=np.float32)
    ctx = np.asarray(ctx, dtype=np.float32)
    Wq = np.asarray(Wq, dtype=np.float32)
    Wk = np.asarray(Wk, dtype=np.float32)
    Wv = np.asarray(Wv, dtype=np.float32)
    Wo = np.asarray(Wo, dtype=np.float32)
    y, _ = run(x, ctx, Wq, Wk, Wv, Wo, trace=False)
    return y


# revision 23
# speedup vs baseline: 1.4354x; 1.0576x over previous
"""Trainium2 Bass kernel for 16-head cross attention, tensor-parallel over 8 cores.

Reference computation (fp32):
    q = (x @ Wq).reshape(n, 16, 64)   # x [2048, 1024], Wq [1024, 1024]
    k = (ctx @ Wk).reshape(m, 16, 64) # ctx [2048, 768]
    v = (ctx @ Wv).reshape(m, 16, 64)
    out[h] = softmax(q[h] @ k[h].T / 8) @ v[h]
    y = out.reshape(n, 1024) @ Wo

Sharding: heads split 2-per-core (columns of Wq/Wk/Wv, rows of Wo). Each core
produces a partial y (transposed); the host sums the 8 partials.

Per-core layout choices:
  - x, ctx are fed pre-transposed (xT [1024, 2048], ctxT [768, 2048]) so the
    contraction dim of every projection matmul lands on SBUF partitions.
  - Scores are computed transposed (scoresT [m, n]) so the PV contraction (m)
    is on partitions; softmax denominators come from a ones-column appended
    to v; no max subtraction (scores ~ N(0,1), exp is safe in fp32).
  - The 1/8 softmax scale is folded into Wq on the host.
"""

import os
import sys

for _p in ("/opt/trn_rl_repo", "/root/.axon_site/_ro/trn_rl_repo"):
    if os.path.isdir(_p) and _p not in sys.path:
        sys.path.insert(0, _p)

import numpy as np
import ml_dtypes

import concourse.bass as bass
import concourse.mybir as mybir
import concourse.tile as tile
from concourse import bacc
from concourse.bass_utils import run_bass_kernel_spmd

P = 128
N_TOK = 2048  # n: query rows
M_TOK = 2048  # m: context rows
D = 1024
C = 768
HEADS = 16
DH = 64  # head dim
HPC = 2  # heads per core
SCALE = 8.0  # sqrt(DH)

NB = 512  # n-block width for the attention phase
DK = D // P  # 8 contraction chunks for x projections
CK = C // P  # 6 contraction chunks for ctx projections
MT = M_TOK // P  # 16 context chunks
NBLK = N_TOK // NB  # 4

# "bf16" | "f32r" | "f32" — f32r keeps ~tf32 precision but its weight loads
# cannot overlap with matmuls (fused 4-byte LDWEIGHTS), costing ~50% PE time.
DTYPE_MODE = os.environ.get("CA_DTYPE", "bf16")


def _dtypes():
    # (storage dtype, numpy dtype, matmul dtype). For f32r the whole chain
    # must be declared float32r (BIR verifier: producers must round to f32r).
    if DTYPE_MODE == "bf16":
        return mybir.dt.bfloat16, ml_dtypes.bfloat16, mybir.dt.bfloat16
    if DTYPE_MODE == "f32r":
        return mybir.dt.float32r, np.float32, mybir.dt.float32r
    return mybir.dt.float32, np.float32, mybir.dt.float32


def _mm_cast(ap, mm_dt):
    return ap.bitcast(mm_dt) if ap.dtype != mm_dt else ap


def build_core_program():
    dt_store, _, dt_mm = _dtypes()
    f32 = mybir.dt.float32

    nc = bacc.Bacc("TRN2", target_bir_lowering=False, debug=False)

    xT = nc.declare_dram_parameter("xT", [D, N_TOK], dt_store, isOutput=False)
    ctxT = nc.declare_dram_parameter("ctxT", [C, M_TOK], dt_store, isOutput=False)
    wq = nc.declare_dram_parameter("wq", [D, P], dt_store, isOutput=False)
    wk = nc.declare_dram_parameter("wk", [C, P], dt_store, isOutput=False)
    wv = nc.declare_dram_parameter("wv", [C, P], dt_store, isOutput=False)
    wo = nc.declare_dram_parameter("wo", [P, D], dt_store, isOutput=False)
    yT = nc.declare_dram_parameter("yT", [D, N_TOK], f32, isOutput=True)

    with tile.TileContext(nc) as tc:
        with (
            tc.tile_pool(name="wts", bufs=1) as wts,
            tc.tile_pool(name="att", bufs=3) as att,
            tc.tile_pool(name="yout", bufs=3) as yout,
            tc.tile_pool(name="small", bufs=4) as small,
            tc.tile_pool(name="ps_big", bufs=2, space="PSUM") as ps_big,  # 2x2 banks
            tc.tile_pool(name="ps_pv", bufs=3, space="PSUM") as ps_pv,  # 3x1
            tc.tile_pool(name="ps_y", bufs=1, space="PSUM") as ps_y,  # 1x1
        ):
            # ---- input DMA (ctx side first: kT and v unblock the attention) ----
            ctxT_sb = wts.tile([P, CK, M_TOK], dt_store)
            for ck in range(CK):
                nc.sync.dma_start(
                    ctxT_sb[:, ck, :], ctxT.ap()[ck * P : (ck + 1) * P, :]
                )
            # weights arrive host-pre-shuffled as [P, o, e] so the DMA is a
            # single contiguous copy (the strided rearrange cost ~15us of
            # descriptor issue on the Sync queue)
            wk_sb = wts.tile([P, CK, P], dt_store)
            nc.sync.dma_start(wk_sb[:], wk.ap().rearrange("(p o) e -> p o e", o=CK))
            wv_sb = wts.tile([P, CK, P], dt_store)
            nc.sync.dma_start(wv_sb[:], wv.ap().rearrange("(p o) e -> p o e", o=CK))
            xT_sb = wts.tile([P, DK, N_TOK], dt_store)
            for dk in range(DK):
                nc.sync.dma_start(xT_sb[:, dk, :], xT.ap()[dk * P : (dk + 1) * P, :])
            wq_sb = wts.tile([P, DK, P], dt_store)
            nc.sync.dma_start(wq_sb[:], wq.ap().rearrange("(p o) e -> p o e", o=DK))
            wo_sb = wts.tile([P, D], dt_store)
            nc.sync.dma_start(wo_sb[:], wo.ap())

            # ---- persistent intermediates ----
            kT_sb = wts.tile([P, N_TOK], dt_store)  # [dk(2 heads), m]
            qT_sb = wts.tile([P, N_TOK], dt_store)  # [dq(2 heads), n]
            vA_sb = wts.tile([P, MT, DH + 1], dt_store)  # [m, mt, dv+ones]
            vB_sb = wts.tile([P, MT, DH + 1], dt_store)
            oT_sb = wts.tile([P, N_TOK], dt_store)  # attn out^T, both heads

            def _memset(ap, val):
                if ap.dtype == mybir.dt.float32r:
                    ap = ap.bitcast(f32)
                nc.vector.memset(ap, val)

            _memset(vA_sb[:, :, DH : DH + 1], 1.0)
            _memset(vB_sb[:, :, DH : DH + 1], 1.0)

            # ---- phase 1a: kT = wk^T @ ctxT  (accumulate over ck) ----
            with nc.named_scope("ph1_kT"):
                for nb in range(4):
                    ps = ps_big.tile([P, 2, NB], f32, tag="ps_big")
                    pk = ps[:, 0, :]
                    for ck in range(CK):
                        nc.tensor.matmul(
                            pk,
                            _mm_cast(wk_sb[:, ck, :], dt_mm),
                            _mm_cast(ctxT_sb[:, ck, nb * NB : (nb + 1) * NB], dt_mm),
                            start=(ck == 0),
                            stop=(ck == CK - 1),
                        )
                    nc.vector.tensor_copy(kT_sb[:, nb * NB : (nb + 1) * NB], pk)

            # ---- phase 1b: qT = wq^T @ xT ----
            with nc.named_scope("ph1_qT"):
                for nb in range(4):
                    ps = ps_big.tile([P, 2, NB], f32, tag="ps_big")
                    pq = ps[:, 0, :]
                    for dk in range(DK):
                        nc.tensor.matmul(
                            pq,
                            _mm_cast(wq_sb[:, dk, :], dt_mm),
                            _mm_cast(xT_sb[:, dk, nb * NB : (nb + 1) * NB], dt_mm),
                            start=(dk == 0),
                            stop=(dk == DK - 1),
                        )
                    nc.vector.tensor_copy(qT_sb[:, nb * NB : (nb + 1) * NB], pq)

            # ---- phase 1c: v natural = ctx @ wv; stationary = ctxT chunk ----
            with nc.named_scope("ph1_v"):
                for mt in range(MT):
                    ps = ps_pv.tile([P, NB], f32, tag="ps_pv")
                    pv = ps[:, :P]
                    for ck in range(CK):
                        nc.tensor.matmul(
                            pv,
                            _mm_cast(ctxT_sb[:, ck, mt * P : (mt + 1) * P], dt_mm),
                            _mm_cast(wv_sb[:, ck, :], dt_mm),
                            start=(ck == 0),
                            stop=(ck == CK - 1),
                        )
                    nc.vector.tensor_copy(vA_sb[:, mt, :DH], pv[:, :DH])
                    nc.vector.tensor_copy(vB_sb[:, mt, :DH], pv[:, DH:])

            # ---- phase 2: attention, with norm/proj of the previous block
            # software-pipelined into the current block's mt loop ----
            def emit_norm(pvA, pvB, nsl):
                # normalize: oT[h] = pv[0:64] * (1/pv[64]) broadcast over rows
                for h, pv in ((0, pvA), (1, pvB)):
                    # recip_approx_fast mishandles nonzero base partitions:
                    # bounce the sums row to a partition-0 SBUF tile first
                    sums_sb = small.tile([1, NB], f32, tag="sums")
                    nc.vector.tensor_copy(sums_sb[:], pv[DH : DH + 1, :])
                    rcf = small.tile([1, NB], f32, tag="recip_f32")
                    nc.vector.reciprocal_approx_fast(rcf[:], sums_sb[:])
                    bcs = small.tile([DH, NB], f32, tag="bcast_sb")
                    nc.gpsimd.partition_broadcast(bcs[:], rcf[:])
                    nc.vector.tensor_mul(
                        oT_sb[h * DH : (h + 1) * DH, nsl], pv[:DH, :], bcs[:]
                    )

            def emit_proj_step(dt_i, nsl):
                # one 128-column slab of yT[dout, nsl]
                py = ps_y.tile([P, NB], f32, tag="ps_y")
                nc.tensor.matmul(
                    py[:],
                    _mm_cast(wo_sb[:, dt_i * P : (dt_i + 1) * P], dt_mm),
                    _mm_cast(oT_sb[:, nsl], dt_mm),
                    start=True,
                    stop=True,
                )
                ys = yout.tile([P, NB], f32, tag="yout")
                nc.vector.tensor_copy(ys[:], py[:])
                nc.sync.dma_start(yT.ap()[dt_i * P : (dt_i + 1) * P, nsl], ys[:])

            prev = None
            for nb in range(NBLK):
                nsl = slice(nb * NB, (nb + 1) * NB)
                with nc.named_scope(f"ph2_att{nb}"):
                    pvA = ps_pv.tile([P, NB], f32, tag="ps_pv")
                    pvB = ps_pv.tile([P, NB], f32, tag="ps_pv")
                    for mt in range(MT):
                        msl = slice(mt * P, (mt + 1) * P)
                        # scoresT for both heads into one 2-bank psum tile
                        sc = ps_big.tile([P, 2, NB], f32, tag="ps_big")
                        nc.tensor.matmul(
                            sc[:, 0, :],
                            _mm_cast(kT_sb[0:DH, msl], dt_mm),
                            _mm_cast(qT_sb[0:DH, nsl], dt_mm),
                            start=True,
                            stop=True,
                        )
                        nc.tensor.matmul(
                            sc[:, 1, :],
                            _mm_cast(kT_sb[DH:P, msl], dt_mm),
                            _mm_cast(qT_sb[DH:P, nsl], dt_mm),
                            start=True,
                            stop=True,
                        )
                        # exp of both heads in one ACT op
                        at = att.tile([P, 2, NB], dt_store, tag="att")
                        nc.scalar.activation(
                            at[:], sc[:], mybir.ActivationFunctionType.Exp
                        )
                        # PV accumulation (ones column gives softmax sums, row DH)
                        nc.tensor.matmul(
                            pvA[: DH + 1, :],
                            _mm_cast(vA_sb[:, mt, :], dt_mm),
                            _mm_cast(at[:, 0, :], dt_mm),
                            start=(mt == 0),
                            stop=(mt == MT - 1),
                        )
                        nc.tensor.matmul(
                            pvB[: DH + 1, :],
                            _mm_cast(vB_sb[:, mt, :], dt_mm),
                            _mm_cast(at[:, 1, :], dt_mm),
                            start=(mt == 0),
                            stop=(mt == MT - 1),
                        )
                        # interleave the previous block's epilogue
                        if prev is not None:
                            if mt == 1:
                                emit_norm(*prev)
                            elif 3 <= mt < 11:
                                emit_proj_step(mt - 3, prev[2])
                prev = (pvA, pvB, nsl)

            with nc.named_scope("ph2_tail"):
                emit_norm(*prev)
                for dt_i in range(8):
                    emit_proj_step(dt_i, prev[2])

    nc.compile()
    return nc


_NC_CACHE = {}


def _get_nc():
    key = DTYPE_MODE
    if key not in _NC_CACHE:
        _NC_CACHE[key] = build_core_program()
    return _NC_CACHE[key]


def _shuffle_w(w):
    # [o*P + p, e] -> [p*o_n + o, e] so each SBUF partition's rows are
    # contiguous in DRAM (single contiguous DMA into a [P, o, e] tile)
    o_n = w.shape[0] // P
    return np.ascontiguousarray(
        w.reshape(o_n, P, w.shape[1]).transpose(1, 0, 2).reshape(w.shape)
    )


def _prep_in_maps(x, ctx, Wq, Wk, Wv, Wo):
    _, np_dt, _ = _dtypes()
    xT = np.ascontiguousarray(x.T).astype(np_dt)
    ctxT = np.ascontiguousarray(ctx.T).astype(np_dt)
    Wq_s = (Wq / SCALE).astype(np.float32)
    in_maps = []
    for cc in range(8):
        csl = slice(cc * P, (cc + 1) * P)
        in_maps.append(
            {
                "xT": xT,
                "ctxT": ctxT,
                "wq": _shuffle_w(np.ascontiguousarray(Wq_s[:, csl])).astype(np_dt),
                "wk": _shuffle_w(np.ascontiguousarray(Wk[:, csl])).astype(np_dt),
                "wv": _shuffle_w(np.ascontiguousarray(Wv[:, csl])).astype(np_dt),
                "wo": np.ascontiguousarray(Wo[csl, :]).astype(np_dt),
            }
        )
    return in_maps


def run(x, ctx, Wq, Wk, Wv, Wo, trace=False):
    nc = _get_nc()
    in_maps = _prep_in_maps(x, ctx, Wq, Wk, Wv, Wo)
    res = run_bass_kernel_spmd(nc, in_maps, core_ids=list(range(8)), trace=trace)
    acc = np.zeros((D, N_TOK), np.float32)
    for r in res.results:
        acc += r["yT"]
    return np.ascontiguousarray(acc.T), res


def kernel(x, ctx, Wq, Wk, Wv, Wo):
    x = np.asarray(x, dtype=np.float32)
    ctx = np.asarray(ctx, dtype=np.float32)
    Wq = np.asarray(Wq, dtype=np.float32)
    Wk = np.asarray(Wk, dtype=np.float32)
    Wv = np.asarray(Wv, dtype=np.float32)
    Wo = np.asarray(Wo, dtype=np.float32)
    y, _ = run(x, ctx, Wq, Wk, Wv, Wo, trace=False)
    return y


# revision 27
# speedup vs baseline: 1.4563x; 1.0145x over previous
"""Trainium2 Bass kernel for 16-head cross attention, tensor-parallel over 8 cores.

Reference computation (fp32):
    q = (x @ Wq).reshape(n, 16, 64)   # x [2048, 1024], Wq [1024, 1024]
    k = (ctx @ Wk).reshape(m, 16, 64) # ctx [2048, 768]
    v = (ctx @ Wv).reshape(m, 16, 64)
    out[h] = softmax(q[h] @ k[h].T / 8) @ v[h]
    y = out.reshape(n, 1024) @ Wo

Sharding: heads split 2-per-core (columns of Wq/Wk/Wv, rows of Wo). Each core
produces a partial y (transposed); the host sums the 8 partials.

Per-core layout choices:
  - x, ctx are fed pre-transposed (xT [1024, 2048], ctxT [768, 2048]) so the
    contraction dim of every projection matmul lands on SBUF partitions.
  - Scores are computed transposed (scoresT [m, n]) so the PV contraction (m)
    is on partitions; softmax denominators come from a ones-column appended
    to v; no max subtraction (scores ~ N(0,1), exp is safe in fp32).
  - The 1/8 softmax scale is folded into Wq on the host.
"""

import os
import sys

for _p in ("/opt/trn_rl_repo", "/root/.axon_site/_ro/trn_rl_repo"):
    if os.path.isdir(_p) and _p not in sys.path:
        sys.path.insert(0, _p)

import numpy as np
import ml_dtypes

import concourse.bass as bass
import concourse.mybir as mybir
import concourse.tile as tile
from concourse import bacc
from concourse.bass_utils import run_bass_kernel_spmd

P = 128
N_TOK = 2048  # n: query rows
M_TOK = 2048  # m: context rows
D = 1024
C = 768
HEADS = 16
DH = 64  # head dim
HPC = 2  # heads per core
SCALE = 8.0  # sqrt(DH)

NB = 512  # n-block width for the attention phase
DK = D // P  # 8 contraction chunks for x projections
CK = C // P  # 6 contraction chunks for ctx projections
MT = M_TOK // P  # 16 context chunks
NBLK = N_TOK // NB  # 4

# "bf16" | "f32r" | "f32" — f32r keeps ~tf32 precision but its weight loads
# cannot overlap with matmuls (fused 4-byte LDWEIGHTS), costing ~50% PE time.
DTYPE_MODE = os.environ.get("CA_DTYPE", "bf16")


def _dtypes():
    # (storage dtype, numpy dtype, matmul dtype). For f32r the whole chain
    # must be declared float32r (BIR verifier: producers must round to f32r).
    if DTYPE_MODE == "bf16":
        return mybir.dt.bfloat16, ml_dtypes.bfloat16, mybir.dt.bfloat16
    if DTYPE_MODE == "f32r":
        return mybir.dt.float32r, np.float32, mybir.dt.float32r
    return mybir.dt.float32, np.float32, mybir.dt.float32


def _mm_cast(ap, mm_dt):
    return ap.bitcast(mm_dt) if ap.dtype != mm_dt else ap


def build_core_program():
    dt_store, _, dt_mm = _dtypes()
    f32 = mybir.dt.float32

    nc = bacc.Bacc("TRN2", target_bir_lowering=False, debug=False)

    xT = nc.declare_dram_parameter("xT", [D, N_TOK], dt_store, isOutput=False)
    ctxT = nc.declare_dram_parameter("ctxT", [C, M_TOK], dt_store, isOutput=False)
    wq = nc.declare_dram_parameter("wq", [D, P], dt_store, isOutput=False)
    wk = nc.declare_dram_parameter("wk", [C, P], dt_store, isOutput=False)
    wv = nc.declare_dram_parameter("wv", [C, P], dt_store, isOutput=False)
    wo = nc.declare_dram_parameter("wo", [P, D], dt_store, isOutput=False)
    yT = nc.declare_dram_parameter("yT", [D, N_TOK], f32, isOutput=True)

    with tile.TileContext(nc) as tc:
        with (
            tc.tile_pool(name="wts", bufs=1) as wts,
            tc.tile_pool(name="att", bufs=3) as att,
            tc.tile_pool(name="yout", bufs=3) as yout,
            tc.tile_pool(name="small", bufs=4) as small,
            tc.tile_pool(name="ps_big", bufs=2, space="PSUM") as ps_big,  # 2x2 banks
            tc.tile_pool(name="ps_pv", bufs=3, space="PSUM") as ps_pv,  # 3x1
            tc.tile_pool(name="ps_y", bufs=1, space="PSUM") as ps_y,  # 1x1
        ):
            # ---- ACT exp-table preload: tiny dummy exp so the ~2.7us
            # ACT_TABLE_LOAD hides under the input DMA ----
            warm = small.tile([1, 8], f32, tag="warm")
            nc.vector.memset(warm[:], 0.0)
            nc.scalar.activation(warm[:], warm[:], mybir.ActivationFunctionType.Exp)

            # ---- input DMA (ctx side first: kT unblocks the attention);
            # weights arrive host-pre-shuffled as [P, o, e] so each DMA is a
            # contiguous copy; issue is spread across the sync/scalar/gpsimd
            # queues so per-DMA descriptor issue (~0.6-1us) doesn't serialize
            ctxT_sb = wts.tile([P, CK, M_TOK], dt_store)
            for ck in range(CK):
                eng = nc.sync if ck % 2 == 0 else nc.scalar
                eng.dma_start(ctxT_sb[:, ck, :], ctxT.ap()[ck * P : (ck + 1) * P, :])
            wk_sb = wts.tile([P, CK, P], dt_store)
            nc.gpsimd.dma_start(wk_sb[:], wk.ap().rearrange("(p o) e -> p o e", o=CK))
            wv_sb = wts.tile([P, CK, P], dt_store)
            nc.gpsimd.dma_start(wv_sb[:], wv.ap().rearrange("(p o) e -> p o e", o=CK))
            xT_sb = wts.tile([P, DK, N_TOK], dt_store)
            for dk in range(DK):
                eng = nc.sync if dk % 2 == 0 else nc.scalar
                eng.dma_start(xT_sb[:, dk, :], xT.ap()[dk * P : (dk + 1) * P, :])
            wq_sb = wts.tile([P, DK, P], dt_store)
            nc.gpsimd.dma_start(wq_sb[:], wq.ap().rearrange("(p o) e -> p o e", o=DK))
            wo_sb = wts.tile([P, D], dt_store)
            nc.gpsimd.dma_start(wo_sb[:], wo.ap())

            # ---- persistent intermediates ----
            kT_sb = wts.tile([P, N_TOK], dt_store)  # [dk(2 heads), m]
            qT_sb = wts.tile([P, N_TOK], dt_store)  # [dq(2 heads), n]
            vA_sb = wts.tile([P, MT, DH + 1], dt_store)  # [m, mt, dv+ones]
            vB_sb = wts.tile([P, MT, DH + 1], dt_store)
            oT_sb = wts.tile([P, N_TOK], dt_store)  # attn out^T, both heads

            def _memset(ap, val):
                if ap.dtype == mybir.dt.float32r:
                    ap = ap.bitcast(f32)
                nc.vector.memset(ap, val)

            _memset(vA_sb[:, :, DH : DH + 1], 1.0)
            _memset(vB_sb[:, :, DH : DH + 1], 1.0)

            # ---- phase 1a: kT = wk^T @ ctxT  (accumulate over ck) ----
            with nc.named_scope("ph1_kT"):
                for nb in range(4):
                    ps = ps_big.tile([P, 2, NB], f32, tag="ps_big")
                    pk = ps[:, 0, :]
                    for ck in range(CK):
                        nc.tensor.matmul(
                            pk,
                            _mm_cast(wk_sb[:, ck, :], dt_mm),
                            _mm_cast(ctxT_sb[:, ck, nb * NB : (nb + 1) * NB], dt_mm),
                            start=(ck == 0),
                            stop=(ck == CK - 1),
                        )
                    nc.vector.tensor_copy(kT_sb[:, nb * NB : (nb + 1) * NB], pk)

            # ---- phase 1b: qT = wq^T @ xT ----
            with nc.named_scope("ph1_qT"):
                for nb in range(4):
                    ps = ps_big.tile([P, 2, NB], f32, tag="ps_big")
                    pq = ps[:, 0, :]
                    for dk in range(DK):
                        nc.tensor.matmul(
                            pq,
                            _mm_cast(wq_sb[:, dk, :], dt_mm),
                            _mm_cast(xT_sb[:, dk, nb * NB : (nb + 1) * NB], dt_mm),
                            start=(dk == 0),
                            stop=(dk == DK - 1),
                        )
                    nc.vector.tensor_copy(qT_sb[:, nb * NB : (nb + 1) * NB], pq)

            # ---- phase 1c: v natural = ctx @ wv; stationary = ctxT chunk ----
            with nc.named_scope("ph1_v"):
                for mt in range(MT):
                    ps = ps_pv.tile([P, NB], f32, tag="ps_pv")
                    pv = ps[:, :P]
                    for ck in range(CK):
                        nc.tensor.matmul(
                            pv,
                            _mm_cast(ctxT_sb[:, ck, mt * P : (mt + 1) * P], dt_mm),
                            _mm_cast(wv_sb[:, ck, :], dt_mm),
                            start=(ck == 0),
                            stop=(ck == CK - 1),
                        )
                    nc.vector.tensor_copy(vA_sb[:, mt, :DH], pv[:, :DH])
                    nc.vector.tensor_copy(vB_sb[:, mt, :DH], pv[:, DH:])

            # ---- phase 2: attention, with norm/proj of the previous block
            # software-pipelined into the current block's mt loop ----
            def emit_norm(pvA, pvB, nsl):
                # normalize: oT[h] = pv[0:64] * (1/pv[64]) broadcast over rows
                for h, pv in ((0, pvA), (1, pvB)):
                    # recip_approx_fast mishandles nonzero base partitions:
                    # bounce the sums row to a partition-0 SBUF tile first
                    sums_sb = small.tile([1, NB], f32, tag="sums")
                    nc.vector.tensor_copy(sums_sb[:], pv[DH : DH + 1, :])
                    rcf = small.tile([1, NB], f32, tag="recip_f32")
                    nc.vector.reciprocal_approx_fast(rcf[:], sums_sb[:])
                    bcs = small.tile([DH, NB], f32, tag="bcast_sb")
                    nc.gpsimd.partition_broadcast(bcs[:], rcf[:])
                    nc.vector.tensor_mul(
                        oT_sb[h * DH : (h + 1) * DH, nsl], pv[:DH, :], bcs[:]
                    )

            def emit_proj_step(dt_i, nsl, tail=False):
                # one 128-column slab of yT[dout, nsl]. In the tail (no att
                # to hide under) pipeline via ps_big slots + alternate the
                # psum->sbuf copy between DVE and the now-idle Scalar engine.
                if tail and dt_i % 2 == 0:
                    py2 = ps_big.tile([P, 2, NB], f32, tag="ps_big", name="py2")
                    py = py2[:, 0, :]
                else:
                    py = ps_y.tile([P, NB], f32, tag="ps_y")
                nc.tensor.matmul(
                    py[:],
                    _mm_cast(wo_sb[:, dt_i * P : (dt_i + 1) * P], dt_mm),
                    _mm_cast(oT_sb[:, nsl], dt_mm),
                    start=True,
                    stop=True,
                )
                ys = yout.tile([P, NB], f32, tag="yout")
                if tail and dt_i % 2 == 1:
                    nc.scalar.copy(ys[:], py[:])
                else:
                    nc.vector.tensor_copy(ys[:], py[:])
                nc.sync.dma_start(yT.ap()[dt_i * P : (dt_i + 1) * P, nsl], ys[:])

            prev = None
            for nb in range(NBLK):
                nsl = slice(nb * NB, (nb + 1) * NB)
                with nc.named_scope(f"ph2_att{nb}"):
                    pvA = ps_pv.tile([P, NB], f32, tag="ps_pv")
                    pvB = ps_pv.tile([P, NB], f32, tag="ps_pv")
                    for mt in range(MT):
                        msl = slice(mt * P, (mt + 1) * P)
                        # scoresT for both heads into one 2-bank psum tile
                        sc = ps_big.tile([P, 2, NB], f32, tag="ps_big")
                        nc.tensor.matmul(
                            sc[:, 0, :],
                            _mm_cast(kT_sb[0:DH, msl], dt_mm),
                            _mm_cast(qT_sb[0:DH, nsl], dt_mm),
                            start=True,
                            stop=True,
                        )
                        nc.tensor.matmul(
                            sc[:, 1, :],
                            _mm_cast(kT_sb[DH:P, msl], dt_mm),
                            _mm_cast(qT_sb[DH:P, nsl], dt_mm),
                            start=True,
                            stop=True,
                        )
                        # exp of both heads in one ACT op
                        at = att.tile([P, 2, NB], dt_store, tag="att")
                        nc.scalar.activation(
                            at[:], sc[:], mybir.ActivationFunctionType.Exp
                        )
                        # PV accumulation (ones column gives softmax sums, row DH)
                        nc.tensor.matmul(
                            pvA[: DH + 1, :],
                            _mm_cast(vA_sb[:, mt, :], dt_mm),
                            _mm_cast(at[:, 0, :], dt_mm),
                            start=(mt == 0),
                            stop=(mt == MT - 1),
                        )
                        nc.tensor.matmul(
                            pvB[: DH + 1, :],
                            _mm_cast(vB_sb[:, mt, :], dt_mm),
                            _mm_cast(at[:, 1, :], dt_mm),
                            start=(mt == 0),
                            stop=(mt == MT - 1),
                        )
                        # interleave the previous block's epilogue
                        if prev is not None:
                            if mt == 1:
                                emit_norm(*prev)
                            elif 3 <= mt < 11:
                                emit_proj_step(mt - 3, prev[2])
                prev = (pvA, pvB, nsl)

            with nc.named_scope("ph2_tail"):
                emit_norm(*prev)
                for dt_i in range(8):
                    emit_proj_step(dt_i, prev[2], tail=True)

    nc.compile()
    return nc


_NC_CACHE = {}


def _get_nc():
    key = DTYPE_MODE
    if key not in _NC_CACHE:
        _NC_CACHE[key] = build_core_program()
    return _NC_CACHE[key]


def _shuffle_w(w):
    # [o*P + p, e] -> [p*o_n + o, e] so each SBUF partition's rows are
    # contiguous in DRAM (single contiguous DMA into a [P, o, e] tile)
    o_n = w.shape[0] // P
    return np.ascontiguousarray(
        w.reshape(o_n, P, w.shape[1]).transpose(1, 0, 2).reshape(w.shape)
    )


def _prep_in_maps(x, ctx, Wq, Wk, Wv, Wo):
    _, np_dt, _ = _dtypes()
    xT = np.ascontiguousarray(x.T).astype(np_dt)
    ctxT = np.ascontiguousarray(ctx.T).astype(np_dt)
    Wq_s = (Wq / SCALE).astype(np.float32)
    in_maps = []
    for cc in range(8):
        csl = slice(cc * P, (cc + 1) * P)
        in_maps.append(
            {
                "xT": xT,
                "ctxT": ctxT,
                "wq": _shuffle_w(np.ascontiguousarray(Wq_s[:, csl])).astype(np_dt),
                "wk": _shuffle_w(np.ascontiguousarray(Wk[:, csl])).astype(np_dt),
                "wv": _shuffle_w(np.ascontiguousarray(Wv[:, csl])).astype(np_dt),
                "wo": np.ascontiguousarray(Wo[csl, :]).astype(np_dt),
            }
        )
    return in_maps


def run(x, ctx, Wq, Wk, Wv, Wo, trace=False):
    nc = _get_nc()
    in_maps = _prep_in_maps(x, ctx, Wq, Wk, Wv, Wo)
    res = run_bass_kernel_spmd(nc, in_maps, core_ids=list(range(8)), trace=trace)
    acc = np.zeros((D, N_TOK), np.float32)
    for r in res.results:
        acc += r["yT"]
    return np.ascontiguousarray(acc.T), res


def kernel(x, ctx, Wq, Wk, Wv, Wo):
    x = np.asarray(x, dtype=np.float32)
    ctx = np.asarray(ctx, dtype=np.float32)
    Wq = np.asarray(Wq, dtype=np.float32)
    Wk = np.asarray(Wk, dtype=np.float32)
    Wv = np.asarray(Wv, dtype=np.float32)
    Wo = np.asarray(Wo, dtype=np.float32)
    y, _ = run(x, ctx, Wq, Wk, Wv, Wo, trace=False)
    return y
